# revision 1
# baseline (speedup 1.0000x reference)
"""TRN2 Bass kernel for a fused multi-head attention block (B=2, N=2048,
C=1024, 16 heads, head_dim 64, per-head q/k LayerNorm, out projection).

Sharding: 8 NeuronCores = 2 (batch) x 4 (head groups of 4 heads).
Each core computes qkv for its 4 heads, per-head LN + attention, and a
partial output projection; the host sums the 4 partials per batch
(tensor-parallel unshard) and adds proj bias.

Per-core dataflow (all matmuls in fp32r, accumulating in fp32 PSUM):
  x[b] -> (PE transpose) x^T -> qkv token-major -> LN(q,k) token-major
  -> (PE transpose) q^T,k^T head-major -> S^T = k_hat @ q_hat^T per head
  -> exp (ACT, scale 1/8) -> O^T(+rowsum) = [v|1]^T @ E^T -> normalize
  -> partial proj = O @ Wp_shard -> DRAM.
"""

import sys

sys.path.insert(0, "/opt/trn_rl_repo")

import numpy as np

# problem shapes (hardcoded; harness contract)
B, NTOK, C = 2, 2048, 1024
NHEADS, HD = 16, 64
EPS = 1e-6
P = 128
KC = C // P  # 8 k-chunks of the C contraction
TCH = NTOK // P  # 16 token chunks
G = NHEADS // 4  # 4 heads per core
GC = G * HD  # 256 cols per section per core
TQ = 512  # tq slab width
NSLAB = NTOK // TQ
SCL = HD**-0.5
GROUPS = [(i, min(i + 2, 16)) for i in range(0, 16, 2)]

PROFILE = False  # set True by test harness to capture NTFF exec time
LAST_RESULTS = None

_CACHE = {}


def _build_nc(has_qkv_bias: bool, ln_affine: bool):
    from contextlib import ExitStack
    from concourse import bacc
    import concourse.tile as tile
    from concourse import mybir
    from concourse.bass import ts
    from concourse.masks import make_identity

    F32 = mybir.dt.float32
    F32R = mybir.dt.float32r
    AX = mybir.AxisListType
    ALU = mybir.AluOpType
    ACTF = mybir.ActivationFunctionType

    from concourse import library_config

    nc = bacc.Bacc("TRN2", target_bir_lowering=False, debug=False)
    x_d = nc.dram_tensor("x_shard", [NTOK, C], F32, kind="ExternalInput")
    wq_d = nc.dram_tensor("wq_shard", [C, 3 * GC], F32, kind="ExternalInput")
    wp_d = nc.dram_tensor("wp_shard", [GC, C], F32, kind="ExternalInput")
    if has_qkv_bias:
        qb_d = nc.dram_tensor("qb_shard", [1, 3 * GC], F32, kind="ExternalInput")
    if ln_affine:
        # rows: [qscale rep4 | kscale rep4], [qbias rep4 | kbias rep4]
        ln_d = nc.dram_tensor("ln_rows", [2, 2 * GC], F32, kind="ExternalInput")
    out_d = nc.dram_tensor("out_part", [NTOK, C], F32, kind="ExternalOutput")

    with tile.TileContext(nc) as tc:
        with ExitStack() as ctx:
            persist = ctx.enter_context(tc.tile_pool(name="persist", bufs=1))
            xT = persist.tile([P, KC, NTOK], F32R, name="xT")
            qkT = persist.tile([P, 4, NTOK], F32R, name="qkT")
            vS = persist.tile([P, TCH, G, HD + 1], F32R, name="vS")
            oT = persist.tile([P, 2, NTOK], F32R, name="oT")
            w_r = persist.tile([P, KC, 3 * GC], F32R, name="w_r")
            wp_r = persist.tile([P, 2, C], F32R, name="wp_r")
            ident = persist.tile([P, P], F32, name="ident")

            nc.gpsimd.load_library(library_config.attn)
            make_identity(nc, ident[:])

            with tc.tile_pool(name="init", bufs=1) as initp:
                t_ones = initp.tile([P, TCH, G], F32, name="t_ones")
                nc.vector.memset(t_ones[:], 1.0)
                nc.vector.tensor_copy(vS[:, :, :, HD], t_ones[:])
                w_f = initp.tile([P, KC, 3 * GC], F32, name="w_f")
                nc.sync.dma_start(w_f[:], wq_d.rearrange("(ko p) c -> p ko c", p=P))
                nc.vector.tensor_copy(w_r[:], w_f[:])
                wp_f = initp.tile([P, 2, C], F32, name="wp_f")
                nc.sync.dma_start(wp_f[:], wp_d.rearrange("(ko p) c -> p ko c", p=P))
                nc.vector.tensor_copy(wp_r[:], wp_f[:])
                if has_qkv_bias:
                    qb1 = initp.tile([1, 3 * GC], F32, name="qb1")
                    nc.sync.dma_start(qb1[:], qb_d[:])
                    brep = persist.tile([P, 3 * GC], F32, name="brep")
                    nc.gpsimd.partition_broadcast(brep[:], qb1[:])
                if ln_affine:
                    ln1 = initp.tile([2, 2 * GC], F32, name="ln1")
                    nc.sync.dma_start(ln1[:], ln_d[:])
                    srep = persist.tile([P, 2 * GC], F32, name="srep")
                    lbrep = persist.tile([P, 2 * GC], F32, name="lbrep")
                    nc.gpsimd.partition_broadcast(srep[:], ln1[0:1, :])
                    nc.gpsimd.partition_broadcast(lbrep[:], ln1[1:2, :])

            # ---- stages 1+2: x^T, qkv, LN, v staging, q/k transposes ----
            with (
                tc.tile_pool(name="s12", bufs=3) as sp12,
                tc.tile_pool(name="tps", bufs=4, space="PSUM") as psT,
                tc.tile_pool(name="qkvps", bufs=2, space="PSUM") as psQK,
                tc.tile_pool(name="stats", bufs=3) as stp,
            ):
                for t in range(TCH):
                    xt = sp12.tile([P, C], F32, tag="xt")
                    nc.sync.dma_start(xt[:], x_d[ts(t, P), :])
                    for kc in range(KC):
                        tp = psT.tile([P, P], F32, tag="tps")
                        nc.tensor.transpose(tp[:], xt[:, ts(kc, P)], ident[:])
                        nc.scalar.copy(xT[:, kc, ts(t, P)], tp[:])

                    psA = psQK.tile([P, 2 * GC], F32, tag="psA")
                    psB = psQK.tile([P, GC], F32, tag="psB")
                    for kc in range(KC):
                        nc.tensor.matmul(
                            psA[:],
                            xT[:, kc, ts(t, P)],
                            w_r[:, kc, 0 : 2 * GC],
                            start=(kc == 0),
                            stop=(kc == KC - 1),
                        )
                        nc.tensor.matmul(
                            psB[:],
                            xT[:, kc, ts(t, P)],
                            w_r[:, kc, 2 * GC : 3 * GC],
                            start=(kc == 0),
                            stop=(kc == KC - 1),
                        )
                    if has_qkv_bias:
                        nc.vector.tensor_tensor(
                            psA[:], psA[:], brep[:, 0 : 2 * GC], ALU.add
                        )
                        nc.vector.tensor_tensor(
                            psB[:], psB[:], brep[:, 2 * GC : 3 * GC], ALU.add
                        )

                    # LayerNorm over head_dim for q|k (8 segments of 64).
                    # Evacuate PSUM first so the banks free up for the next
                    # chunk's matmuls while the LN chain runs from SBUF.
                    qkA = sp12.tile([P, 2 * GC], F32, tag="qkA")
                    nc.scalar.copy(qkA[:], psA[:])
                    a3 = qkA[:].rearrange("p (g d) -> p g d", d=HD)
                    sq = sp12.tile([P, 2 * GC], F32, tag="sq")
                    nc.scalar.square(sq[:], qkA[:])
                    sums = stp.tile([P, 8], F32, tag="sums")
                    nc.vector.tensor_reduce(sums[:], a3, axis=AX.X, op=ALU.add)
                    sumsq = stp.tile([P, 8], F32, tag="sumsq")
                    nc.vector.tensor_reduce(
                        sumsq[:],
                        sq[:].rearrange("p (g d) -> p g d", d=HD),
                        axis=AX.X,
                        op=ALU.add,
                    )
                    mean = stp.tile([P, 8], F32, tag="mean")
                    nc.vector.tensor_scalar_mul(mean[:], sums[:], 1.0 / HD)
                    msq = stp.tile([P, 8], F32, tag="msq")
                    nc.vector.tensor_tensor(msq[:], mean[:], mean[:], ALU.mult)
                    varep = stp.tile([P, 8], F32, tag="varep")
                    nc.vector.scalar_tensor_tensor(
                        varep[:],
                        in0=sumsq[:],
                        scalar=1.0 / HD,
                        in1=msq[:],
                        op0=ALU.mult,
                        op1=ALU.subtract,
                    )
                    nc.vector.tensor_scalar_add(varep[:], varep[:], EPS)
                    rvar = stp.tile([P, 8], F32, tag="rvar")
                    nc.vector.reciprocal(rvar[:], varep[:])
                    rstd = stp.tile([P, 8], F32, tag="rstd")
                    nc.scalar.activation(rstd[:], rvar[:], ACTF.Sqrt)
                    nmr = stp.tile([P, 8], F32, tag="nmr")
                    nc.vector.scalar_tensor_tensor(
                        nmr[:],
                        in0=mean[:],
                        scalar=-1.0,
                        in1=rstd[:],
                        op0=ALU.mult,
                        op1=ALU.mult,
                    )
                    qkl = sp12.tile([P, 2 * GC], F32, tag="qkl")
                    q3 = qkl[:].rearrange("p (g d) -> p g d", d=HD)
                    nc.vector.tensor_tensor(
                        q3, a3, rstd[:, :, None].to_broadcast([P, 8, HD]), ALU.mult
                    )
                    nc.vector.tensor_tensor(
                        q3, q3, nmr[:, :, None].to_broadcast([P, 8, HD]), ALU.add
                    )
                    if ln_affine:
                        nc.vector.tensor_tensor(qkl[:], qkl[:], srep[:], ALU.mult)
                        nc.vector.tensor_tensor(qkl[:], qkl[:], lbrep[:], ALU.add)

                    # v staging (rounded to f32r), [tok, head, hd]
                    nc.scalar.copy(
                        vS[:, t, :, 0:HD],
                        psB[:].rearrange("p (g d) -> p g d", d=HD),
                    )
                    # transpose LN'd q,k into head-major [hd, tok] (2 heads/tile)
                    for pr in range(4):
                        tp2 = psT.tile([P, P], F32, tag="tps")
                        nc.tensor.transpose(tp2[:], qkl[:, ts(pr, P)], ident[:])
                        if pr % 2:
                            nc.scalar.copy(qkT[:, pr, ts(t, P)], tp2[:])
                        else:
                            nc.vector.tensor_copy(qkT[:, pr, ts(t, P)], tp2[:])

            # ---- stage 3: attention per head / tq slab ----
            # Single-head stream, 2-chunk S groups, 3-deep PSUM pipelining
            # (spt bufs=3) so the PE always has a ready group while ACT exps.
            with (
                tc.tile_pool(name="s3e", bufs=3) as ep,
                tc.tile_pool(name="s3r", bufs=2) as rp,
                tc.tile_pool(name="sps", bufs=3, space="PSUM") as sps,
                tc.tile_pool(name="ops", bufs=2, space="PSUM") as ops,
            ):
                for h in range(G):
                    pr = h // 2
                    pb = (h % 2) * HD
                    for s in range(NSLAB):
                        osum = ops.tile([HD + 1, TQ], F32, tag="osum")
                        for k0, k1 in GROUPS:
                            glen = k1 - k0
                            spt = sps.tile([P, 2, TQ], F32, tag="spt")
                            for j in range(glen):
                                tk = k0 + j
                                nc.tensor.matmul(
                                    spt[:, j],
                                    qkT[pb : pb + HD, 2 + pr, ts(tk, P)],
                                    qkT[pb : pb + HD, pr, ts(s, TQ)],
                                    start=True,
                                    stop=True,
                                )
                            et = ep.tile([P, 2, TQ], F32R, tag="et")
                            nc.scalar.activation(
                                et[:, 0:glen], spt[:, 0:glen], ACTF.Exp, scale=SCL
                            )
                            for j in range(glen):
                                tk = k0 + j
                                nc.tensor.matmul(
                                    osum[:],
                                    vS[:, tk, h, :],
                                    et[:, j],
                                    start=(tk == 0),
                                    stop=(tk == TCH - 1),
                                )
                        rec = rp.tile([1, TQ], F32, tag="rec")
                        nc.vector.reciprocal(rec[:], osum[HD : HD + 1, :])
                        bcr = rp.tile([HD, TQ], F32, tag="bcr")
                        nc.gpsimd.partition_broadcast(bcr[:], rec[:])
                        nc.vector.tensor_tensor(
                            oT[pb : pb + HD, pr, ts(s, TQ)],
                            osum[0:HD, :],
                            bcr[:],
                            ALU.mult,
                        )

            # ---- stage 4: partial output projection ----
            with (
                tc.tile_pool(name="s4", bufs=3) as s4p,
                tc.tile_pool(name="pps", bufs=2, space="PSUM") as pps,
            ):
                for t in range(TCH):
                    for n2 in range(2):
                        pp = pps.tile([P, 512], F32, tag="pp")
                        for kc2 in range(2):
                            nc.tensor.matmul(
                                pp[:],
                                oT[:, kc2, ts(t, P)],
                                wp_r[:, kc2, ts(n2, 512)],
                                start=(kc2 == 0),
                                stop=(kc2 == 1),
                            )
                        ob = s4p.tile([P, 512], F32, tag="ob")
                        nc.scalar.copy(ob[:], pp[:])
                        nc.sync.dma_start(out_d[ts(t, P), ts(n2, 512)], ob[:])

    nc.compile()
    return nc


def _get_nc(has_qkv_bias: bool, ln_affine: bool):
    key = (has_qkv_bias, ln_affine)
    if key not in _CACHE:
        _CACHE[key] = _build_nc(*key)
    return _CACHE[key]


def kernel(**inputs) -> np.ndarray:
    global LAST_RESULTS
    from concourse.bass_utils import run_bass_kernel_spmd

    x = np.asarray(inputs["x"], dtype=np.float32)
    qkv_w = np.asarray(inputs["qkv_w"], dtype=np.float32)
    qkv_b = np.asarray(inputs["qkv_b"], dtype=np.float32)
    qn_scale = np.asarray(inputs["qn_scale"], dtype=np.float32)
    qn_bias = np.asarray(inputs["qn_bias"], dtype=np.float32)
    kn_scale = np.asarray(inputs["kn_scale"], dtype=np.float32)
    kn_bias = np.asarray(inputs["kn_bias"], dtype=np.float32)
    proj_w = np.asarray(inputs["proj_w"], dtype=np.float32)
    proj_b = np.asarray(inputs["proj_b"], dtype=np.float32)

    has_qkv_bias = bool(np.any(qkv_b != 0))
    ln_affine = not (
        np.all(qn_scale == 1)
        and np.all(kn_scale == 1)
        and np.all(qn_bias == 0)
        and np.all(kn_bias == 0)
    )
    nc = _get_nc(has_qkv_bias, ln_affine)

    in_maps = []
    for c in range(8):
        b, g = divmod(c, 4)
        cs = slice(g * GC, (g + 1) * GC)
        wq = np.ascontiguousarray(
            np.concatenate(
                [qkv_w[:, cs], qkv_w[:, C:][:, cs], qkv_w[:, 2 * C :][:, cs]], axis=1
            )
        )
        m = {
            "x_shard": np.ascontiguousarray(x[b]),
            "wq_shard": wq,
            "wp_shard": np.ascontiguousarray(proj_w[cs, :]),
        }
        if has_qkv_bias:
            m["qb_shard"] = np.concatenate(
                [qkv_b[cs], qkv_b[C:][cs], qkv_b[2 * C :][cs]]
            ).reshape(1, 3 * GC)
        if ln_affine:
            m["ln_rows"] = np.stack(
                [
                    np.concatenate([np.tile(qn_scale, G), np.tile(kn_scale, G)]),
                    np.concatenate([np.tile(qn_bias, G), np.tile(kn_bias, G)]),
                ]
            ).astype(np.float32)
        in_maps.append(m)

    res = run_bass_kernel_spmd(
        nc, in_maps, core_ids=list(range(8)), trace=PROFILE
    )
    LAST_RESULTS = res

    out = np.empty((B, NTOK, C), dtype=np.float32)
    for b in range(B):
        acc = res.results[4 * b]["out_part"].astype(np.float32).copy()
        for g in range(1, 4):
            acc += res.results[4 * b + g]["out_part"]
        out[b] = acc + proj_b[None, :]
    return out

